# revision 4
# baseline (speedup 1.0000x reference)
"""Trainium2 Bass kernel for nn_MinimalBeatDecoder (nms_detection), v2.

Reference semantics: peaks = positive local maxima of a 7-wide window over a
16.7M-frame logit stream; output = frame index of each peak (sections are
single peaks in the no-tie case), first 2^21 of them, padded with -1.

v2 strategy (per core, 2^21 frames as 128 rows x 16384, 8 chunks of 2048):
  - ACT engine casts each fp32 chunk into two dense bf16 planes (even/odd
    frames) via strided activation copies; bf16 rounding is monotone, so
    bf16 comparisons give a candidate-peak SUPERSET of the true fp32 peaks.
  - DVE computes the 7-window peak mask on the bf16 planes with dense
    2x-mode tensor_tensor ops (6-window max Q via one 1x STT, then per-
    parity edge max + is_ge), ~6.5us per chunk vs ~12.6us for the fp32 v1.
  - a custom DVE op (BEAT_SCANIDX, registered at import) fuses candidate
    merge + rank scan + scatter-index + per-chunk count into ONE 1x pass.
  - GPSIMD local_scatter compacts candidate positions into 384-slot buckets
    per (row, chunk); buckets are pre-filled with -1 sentinels so rare
    same-pair double candidates (bf16 ties) leave a recoverable hole.
  - host: exact fp32 verification of every candidate (vectorized window max
    at candidate positions) removes bf16 false positives; then unshard.

An exact numpy fallback handles inputs with adjacent-equal fp32 peak ties
(impossible for the gaussian test input, but kept for safety).
"""

import sys

sys.path.insert(0, "/opt/trn_rl_repo")

import numpy as np

import concourse.bacc as bacc
import concourse.bass as bass
import concourse.mybir as mybir
import concourse.tile as tile
from concourse import bass_utils
from concourse import dve_ops
from concourse.dve_spec import (
    Spec, Src0, Src1, AluOp, Zero, One, C0, scan, select, lower,
)
from concourse.dve_spec import _has_src1 as _has_src1
from concourse.dve_uop import DveOpSpec

# geometry
NCORES = 8
NFRAMES = 16_777_216
PERCORE = NFRAMES // NCORES  # 2^21
MAX_BEATS = NFRAMES // 8  # 2^21

P = 128
W = PERCORE // P  # 16384 frames per row
CW = 2048  # chunk width (frames per row per chunk)
HW = CW // 2  # pairs per chunk row
NCH = W // CW  # 8 chunks
K = 384  # bucket slots per (row, chunk); max real count ~330
HALO = 8

F32 = mybir.dt.float32
BF16 = mybir.dt.bfloat16
I16 = mybir.dt.int16

EPS_POS = 1e-38  # positive threshold folded into the window max


def _register_op(name, spec, subdim=False):
    for op in dve_ops.OPS:
        if op.name == name:
            return op
    row = dve_ops._CUSTOM_DVE_ROW_BASE + len(dve_ops.OPS)
    assert row < 0x20
    shas = {}
    for ver in ("v3", "v4"):
        try:
            uops = lower(spec, ver=ver)
            shas[ver] = DveOpSpec(
                name=name, opcode=row, uops=uops, rd1_en=_has_src1(spec)
            ).sha(ver)
        except Exception:
            pass
    op = dve_ops.DveOp(name, spec, subdim=subdim, uops_sha=shas)
    dve_ops.OPS.append(op)
    dve_ops.CUSTOM_DVE_SPECS[name] = spec
    dve_ops._SUB_OPCODE_FOR_NAME[name] = row
    return op


# pk = in0 + in1 (candidates per pair; may be 2 on rare bf16 ties);
# r = inclusive running sum; out = r-1 at candidate pairs else -1;
# accum_out = max(out) = count-1  (s0 = -1.0 seeds the accumulator)
_pk = Src0 + Src1
_r = scan(AluOp.ADD, _pk)
SCANIDX = _register_op(
    "BEAT_SCANIDX",
    Spec(
        body=select(_pk, _r, Zero) - One,
        accum=AluOp.MAX,
        accum_init=C0,
        reference=lambda in0, in1, s0: np.where(
            (in0 + in1) > 0, np.cumsum(in0 + in1, axis=-1), 0
        )
        - 1,
    ),
)


def build_kernel(p=P, w=W):
    """Per-core SPMD program. Inputs:
      xin [p*w + HALO] f32  (frame t of this core at index t+4)
    Outputs:
      stage  [p, NCH*K] i16 (bucketed local pair positions, -1 sentinels)
      counts [p, NCH] f32   (candidates per (row, chunk) minus 1)
    """
    nc = bacc.Bacc("TRN2", target_bir_lowering=False)
    xin = nc.dram_tensor("xin", [p * w + HALO], F32, kind="ExternalInput")
    stage = nc.dram_tensor("stage", [p, NCH * K], I16, kind="ExternalOutput")
    counts = nc.dram_tensor("counts", [p, NCH], F32, kind="ExternalOutput")

    MX = mybir.AluOpType.max
    GE = mybir.AluOpType.is_ge
    ADD = mybir.AluOpType.add

    with tile.TileContext(nc) as tc:
        with (
            tc.tile_pool(name="io", bufs=3) as io_pool,
            tc.tile_pool(name="pl", bufs=3) as pl_pool,
            tc.tile_pool(name="wk", bufs=2) as wk_pool,
            tc.tile_pool(name="st", bufs=3) as st_pool,
            tc.tile_pool(name="acc", bufs=1) as acc_pool,
        ):
            iota2 = acc_pool.tile([p, HW], I16)  # 0,2,4,...
            nc.gpsimd.iota(iota2[:], pattern=[[2, HW]], channel_multiplier=0)
            cntf = acc_pool.tile([p, NCH], F32)

            for j in range(NCH):
                off = j * CW
                # fp32 chunk with +-4 halo; row r reads xin[r*w+off .. +CW+8)
                xh = io_pool.tile([p, CW + HALO], F32, tag="xh")
                src = bass.AP(tensor=xin, offset=off, ap=[[w, p], [1, CW + HALO]])
                nc.sync.dma_start(xh[:], src)

                # bf16 planes: xet[k] = bf16(x[2(k-2)]), xot[k] = bf16(x[2(k-2)+1])
                # (chunk-local frame indices; k in [0, HW+4))
                xet = pl_pool.tile([p, HW + 4], BF16, tag="xet")
                xot = pl_pool.tile([p, HW + 4], BF16, tag="xot")
                nc.scalar.activation(
                    xet[:], xh[:, 0 : CW + 8 : 2],
                    mybir.ActivationFunctionType.Copy, bias=0.0,
                )
                nc.scalar.activation(
                    xot[:], xh[:, 1 : CW + 8 : 2],
                    mybir.ActivationFunctionType.Copy, bias=0.0,
                )

                # pair max P[s] = max(x[2s], x[2s+1]); Pt[k] holds P[k-2]
                Pt = wk_pool.tile([p, HW + 4], BF16, tag="Pt")
                nc.vector.tensor_tensor(out=Pt[:], in0=xet[:], in1=xot[:], op=MX)
                # R[s] = max(P[s-1], P[s+1])
                Rt = wk_pool.tile([p, HW], BF16, tag="Rt")
                nc.vector.tensor_tensor(
                    out=Rt[:], in0=Pt[:, 1 : HW + 1], in1=Pt[:, 3 : HW + 3], op=MX
                )
                # Q[s] = max(R[s], eps, P[s]) = max(x[2s-2..2s+3], eps)
                Qt = wk_pool.tile([p, HW], BF16, tag="Qt")
                nc.vector.scalar_tensor_tensor(
                    out=Qt[:], in0=Rt[:], scalar=EPS_POS, in1=Pt[:, 2 : HW + 2],
                    op0=MX, op1=MX,
                )
                # even: W7 = max(Q[s], x[2s-3]) ; cand = xe >= W7
                eW = wk_pool.tile([p, HW], BF16, tag="eW")
                nc.vector.tensor_tensor(
                    out=eW[:], in0=Qt[:], in1=xot[:, 0:HW], op=MX
                )
                eM = wk_pool.tile([p, HW], I16, tag="eM")
                nc.vector.tensor_tensor(
                    out=eM[:], in0=xet[:, 2 : HW + 2], in1=eW[:], op=GE
                )
                # odd: W7 = max(Q[s], x[2s+4]) ; cand = xo >= W7
                oW = wk_pool.tile([p, HW], BF16, tag="oW")
                nc.vector.tensor_tensor(
                    out=oW[:], in0=Qt[:], in1=xet[:, 4 : HW + 4], op=MX
                )
                oM = wk_pool.tile([p, HW], I16, tag="oM")
                nc.vector.tensor_tensor(
                    out=oM[:], in0=xot[:, 2 : HW + 2], in1=oW[:], op=GE
                )

                # payload: local frame position = 2s + oM
                pay2 = wk_pool.tile([p, HW], I16, tag="pay2")
                nc.vector.tensor_tensor(out=pay2[:], in0=iota2[:], in1=oM[:], op=ADD)
                # fused rank/index + count
                idx16 = wk_pool.tile([p, HW], I16, tag="idx16")
                nc.vector._custom_dve(
                    SCANIDX, out=idx16[:], in0=eM[:], in1=oM[:], s0=-1.0,
                    accum_out=cntf[:, j : j + 1],
                )

                # compact into bucket (LocalScatter zero-fills unwritten
                # slots, so a rare same-pair double candidate leaves a 0
                # "hole"; a legit payload 0 can only sit in slot 0)
                bkt = st_pool.tile([p, K], I16, tag="bkt")
                nc.gpsimd.local_scatter(
                    out_ap=bkt[:], data_ap=pay2[:], idxs_ap=idx16[:],
                    channels=p, num_elems=K, num_idxs=HW,
                )
                nc.gpsimd.dma_start(stage[:, j * K : (j + 1) * K], bkt[:])

            nc.scalar.dma_start(counts[:], cntf[:])
    nc.compile()
    return nc


_cached = {}


def _get_nc():
    if "nc" not in _cached:
        _cached["nc"] = build_kernel()
    return _cached["nc"]


def _host_reference_fallback(x):
    """Exact numpy fallback (used only for adjacent-equal fp32 peak ties)."""
    n = x.shape[0]
    import numpy.lib.stride_tricks as st

    xp = np.pad(x, (3, 3), constant_values=-np.inf)
    pooled = st.sliding_window_view(xp, 7).max(axis=1)
    peak = (x == pooled) & (x > 0)
    idx = np.arange(n, dtype=np.int64)
    prev = np.concatenate([[False], peak[:-1]])
    is_new = peak & ~prev
    sec = np.cumsum(is_new) - 1
    sums = np.zeros(MAX_BEATS + 1, np.float64)
    cnts = np.zeros(MAX_BEATS + 1, np.float64)
    sel = peak & (sec < MAX_BEATS)
    np.add.at(sums, sec[sel], idx[sel].astype(np.float64))
    np.add.at(cnts, sec[sel], 1.0)
    out = np.full(MAX_BEATS, -1.0, np.float32)
    m = cnts[:MAX_BEATS] > 0
    out[m] = (sums[:MAX_BEATS][m] / cnts[:MAX_BEATS][m]).astype(np.float32)
    return out[None, :]


def kernel(logit: np.ndarray) -> np.ndarray:
    x = np.asarray(logit, dtype=np.float32)[0]

    # host guard: adjacent-equal fp32 window maxima need the exact path
    eq_next = x[:-1] == x[1:]
    if eq_next.any():
        cand = np.nonzero(eq_next)[0]
        cand = cand[(x[cand] > 0)]
        if cand.size:
            xp = np.pad(x, (3, 3), constant_values=-np.inf)
            for i in cand:
                if (
                    x[i] == xp[i : i + 7].max()
                    and x[i + 1] == xp[i + 1 : i + 8].max()
                ):
                    return _host_reference_fallback(x)

    nc = _get_nc()

    xpad = np.full(NFRAMES + 8, np.float32(-3.0e38), dtype=np.float32)
    xpad[4 : 4 + NFRAMES] = x

    in_maps = []
    for c in range(NCORES):
        base = c * PERCORE
        in_maps.append(
            {"xin": np.ascontiguousarray(xpad[base : base + PERCORE + HALO])}
        )

    global _last_in_maps
    _last_in_maps = in_maps
    res = bass_utils.run_bass_kernel_spmd(nc, in_maps, core_ids=list(range(NCORES)))

    # host: decode buckets -> candidate positions (global, ascending)
    cand_parts = []
    for c in range(NCORES):
        S = res.results[c]["stage"].reshape(P, NCH, K).astype(np.int64)
        cnt = res.results[c]["counts"]  # [P, NCH] f32, count-1
        C = np.clip(cnt.astype(np.int64) + 1, 0, K)
        # a 0 in slot k>0 (or in slot 0 with slot 1 == 1) is a hole left by
        # a same-pair double candidate: the next slot holds the odd
        # position, so the hole's even position is that value - 1
        nxt = np.roll(S, -1, axis=2)
        hole = S == 0
        hole[:, :, 0] &= nxt[:, :, 0] == 1
        S = np.where(hole, nxt - 1, S)
        base = (
            c * PERCORE
            + np.arange(P, dtype=np.int64)[:, None, None] * W
            + np.arange(NCH, dtype=np.int64)[None, :, None] * CW
        )
        G = S + base  # [P, NCH, K] global positions
        valid = np.arange(K, dtype=np.int64)[None, None, :] < C[:, :, None]
        cand_parts.append(G[valid])
    cand = np.concatenate(cand_parts)

    # exact fp32 verification of every candidate (removes bf16 ties)
    xg = np.pad(x, (3, 3), constant_values=-np.float32(np.inf))
    win = xg[cand[:, None] + np.arange(7)[None, :]]  # cand+3 centers in xg
    xv = x[cand]
    keep = (xv >= win.max(axis=1)) & (xv > 0)
    beats = cand[keep][:MAX_BEATS]

    out = np.full(MAX_BEATS, -1.0, dtype=np.float32)
    out[: beats.size] = beats.astype(np.float32)
    return out[None, :]


# revision 6
# speedup vs baseline: 1.0139x; 1.0139x over previous
"""Trainium2 Bass kernel for nn_MinimalBeatDecoder (nms_detection), v2.2.

Reference semantics: peaks = positive local maxima of a 7-wide window over a
16.7M-frame logit stream; output = frame index of each peak (sections are
single peaks in the no-tie case), first 2^21 of them, padded with -1.

Per core (2^21 frames as 128 rows x 16384), chunked [512, 1536, 2048 x 7]:
  - ACT engine casts each fp32 chunk into two dense bf16 planes (even/odd
    frames) via strided activation copies; bf16 rounding is monotone, so
    bf16 comparisons give a candidate-peak SUPERSET of the true fp32 peaks.
  - DVE computes the 7-window peak mask on the bf16 planes with dense
    2x-mode tensor_tensor ops (P/R + one 1x STT for the 6-window max Q,
    then per-parity edge max + is_ge).
  - custom DVE op BEAT_SCANIDX2 (registered at import) fuses candidate
    merge + REGION-CONTINUED rank scan (init from a per-partition AP) +
    scatter-index + running count in ONE 1x pass; a tiny ACT op chains the
    count into the next chunk's scan seed.
  - pay/idx streams accumulate across chunks of a region; ONE GPSIMD
    local_scatter per region (4096/3072/1024 pairs) compacts into a
    1536/1152/384-slot bucket. LocalScatter starves the DVE while it runs,
    so fewer/larger scatters minimize the serialized time; the small last
    region keeps the pipeline tail short.
  - host: exact fp32 verification of every candidate removes bf16 false
    positives; scatter zero-fill leaves a 0 "hole" for rare same-pair
    double candidates (recovered from the following slot).

An exact numpy fallback handles inputs with adjacent-equal fp32 peak ties.
"""

import sys

sys.path.insert(0, "/opt/trn_rl_repo")

import numpy as np

import concourse.bacc as bacc
import concourse.bass as bass
import concourse.mybir as mybir
import concourse.tile as tile
from concourse import bass_utils
from concourse import dve_ops
from concourse.dve_spec import (
    Spec, Src0, Src1, AluOp, Zero, One, C0, C1, scan, select, lower,
)
from concourse.dve_spec import _has_src1 as _has_src1
from concourse.dve_uop import DveOpSpec

# geometry
NCORES = 8
NFRAMES = 16_777_216
PERCORE = NFRAMES // NCORES  # 2^21
MAX_BEATS = NFRAMES // 8  # 2^21

P = 128
W = PERCORE // P  # 16384 frames per row
HALO = 8

# chunk widths (frames) and region grouping (pairs accumulate per region,
# one scatter per region)
CHUNKS = [512, 1536, 2048, 2048, 2048, 2048, 2048, 2048, 2048]
NCH = len(CHUNKS)
REGIONS = [
    {"chunks": [0, 1, 2, 3, 4], "K": 1536},  # frames [0, 8192)
    {"chunks": [5, 6, 7], "K": 1152},        # frames [8192, 14336)
    {"chunks": [8], "K": 384},               # frames [14336, 16384)
]
RGPAIRS = 4096  # pay/idx buffer width (pairs) per region
STAGE_W = sum(r["K"] for r in REGIONS)

F32 = mybir.dt.float32
BF16 = mybir.dt.bfloat16
I16 = mybir.dt.int16

EPS_POS = 1e-38


def _register_op(name, spec, subdim=False):
    for op in dve_ops.OPS:
        if op.name == name:
            return op
    row = dve_ops._CUSTOM_DVE_ROW_BASE + len(dve_ops.OPS)
    assert row < 0x20
    shas = {}
    for ver in ("v3", "v4"):
        try:
            uops = lower(spec, ver=ver)
            shas[ver] = DveOpSpec(
                name=name, opcode=row, uops=uops, rd1_en=_has_src1(spec)
            ).sha(ver)
        except Exception:
            pass
    op = dve_ops.DveOp(name, spec, subdim=subdim, uops_sha=shas)
    dve_ops.OPS.append(op)
    dve_ops.CUSTOM_DVE_SPECS[name] = spec
    dve_ops._SUB_OPCODE_FOR_NAME[name] = row
    return op


# pk = in0 + in1 in {0,1,2}; r = s1 + inclusive running sum (region-chained);
# out = r-1 at candidate pairs else -1; accum_out = max(out, s0) = count-1
_pk = Src0 + Src1
_r = scan(AluOp.ADD, _pk, init=C1)
SCANIDX2 = _register_op(
    "BEAT_SCANIDX2",
    Spec(
        body=select(_pk, _r, Zero) - One,
        accum=AluOp.MAX,
        accum_init=C0,
        reference=lambda in0, in1, s0, s1: np.where(
            (in0 + in1) > 0, s1 + np.cumsum(in0 + in1, axis=-1), 0
        )
        - 1,
    ),
)


def build_kernel(p=P, w=W):
    """Per-core SPMD program. Inputs:
      xin [p*w + HALO] f32  (frame t of this core at index t+4)
    Outputs:
      stage  [p, STAGE_W] i16 (bucketed region-local pair positions)
      counts [p, NCH] f32     (cumulative-in-region candidate count - 1)
    """
    nc = bacc.Bacc("TRN2", target_bir_lowering=False)
    xin = nc.dram_tensor("xin", [p * w + HALO], F32, kind="ExternalInput")
    stage = nc.dram_tensor("stage", [p, STAGE_W], I16, kind="ExternalOutput")
    counts = nc.dram_tensor("counts", [p, NCH], F32, kind="ExternalOutput")

    MX = mybir.AluOpType.max
    GE = mybir.AluOpType.is_ge
    ADD = mybir.AluOpType.add
    COPY = mybir.ActivationFunctionType.Copy

    with tile.TileContext(nc) as tc:
        with (
            tc.tile_pool(name="io", bufs=3) as io_pool,
            tc.tile_pool(name="pl", bufs=3) as pl_pool,
            tc.tile_pool(name="wk", bufs=2) as wk_pool,
            tc.tile_pool(name="rg", bufs=2) as rg_pool,
            tc.tile_pool(name="st", bufs=2) as st_pool,
            tc.tile_pool(name="acc", bufs=1) as acc_pool,
        ):
            iotaR = acc_pool.tile([p, RGPAIRS], I16)  # 0,2,...,2*RGPAIRS-2
            nc.gpsimd.iota(iotaR[:], pattern=[[2, RGPAIRS]], channel_multiplier=0)
            zero1 = acc_pool.tile([p, 1], F32)
            nc.gpsimd.memset(zero1[:], 0)
            cntf = acc_pool.tile([p, NCH], F32)
            r0 = acc_pool.tile([p, NCH], F32)  # per-chunk scan seeds

            koff = 0
            for ri, reg in enumerate(REGIONS):
                PAY = rg_pool.tile([p, RGPAIRS], I16, tag="PAY")
                IDX = rg_pool.tile([p, RGPAIRS], I16, tag="IDX")
                o = 0  # pair offset within region
                for ci, j in enumerate(reg["chunks"]):
                    cw = CHUNKS[j]
                    hw = cw // 2
                    off = sum(CHUNKS[:j])
                    xh = io_pool.tile([p, 2056], F32, tag="xh")
                    src = bass.AP(tensor=xin, offset=off, ap=[[w, p], [1, cw + 8]])
                    nc.sync.dma_start(xh[:, 0 : cw + 8], src)

                    # bf16 planes: xet[k]=bf16(x[2(k-2)]), xot[k]=bf16(x[2(k-2)+1])
                    xet = pl_pool.tile([p, 1028], BF16, tag="xet")
                    xot = pl_pool.tile([p, 1028], BF16, tag="xot")
                    nc.scalar.activation(
                        xet[:, 0 : hw + 4], xh[:, 0 : cw + 8 : 2], COPY, bias=0.0
                    )
                    nc.scalar.activation(
                        xot[:, 0 : hw + 4], xh[:, 1 : cw + 8 : 2], COPY, bias=0.0
                    )

                    Pt = wk_pool.tile([p, 1028], BF16, tag="Pt")
                    nc.vector.tensor_tensor(
                        out=Pt[:, 0 : hw + 4], in0=xet[:, 0 : hw + 4],
                        in1=xot[:, 0 : hw + 4], op=MX,
                    )
                    Rt = wk_pool.tile([p, 1024], BF16, tag="Rt")
                    nc.vector.tensor_tensor(
                        out=Rt[:, 0:hw], in0=Pt[:, 1 : hw + 1],
                        in1=Pt[:, 3 : hw + 3], op=MX,
                    )
                    Qt = wk_pool.tile([p, 1024], BF16, tag="Qt")
                    nc.vector.scalar_tensor_tensor(
                        out=Qt[:, 0:hw], in0=Rt[:, 0:hw], scalar=EPS_POS,
                        in1=Pt[:, 2 : hw + 2], op0=MX, op1=MX,
                    )
                    eW = wk_pool.tile([p, 1024], BF16, tag="eW")
                    nc.vector.tensor_tensor(
                        out=eW[:, 0:hw], in0=Qt[:, 0:hw], in1=xot[:, 0:hw], op=MX
                    )
                    eM = wk_pool.tile([p, 1024], I16, tag="eM")
                    nc.vector.tensor_tensor(
                        out=eM[:, 0:hw], in0=xet[:, 2 : hw + 2], in1=eW[:, 0:hw],
                        op=GE,
                    )
                    oW = wk_pool.tile([p, 1024], BF16, tag="oW")
                    nc.vector.tensor_tensor(
                        out=oW[:, 0:hw], in0=Qt[:, 0:hw], in1=xet[:, 4 : hw + 4],
                        op=MX,
                    )
                    oM = wk_pool.tile([p, 1024], I16, tag="oM")
                    nc.vector.tensor_tensor(
                        out=oM[:, 0:hw], in0=xot[:, 2 : hw + 2], in1=oW[:, 0:hw],
                        op=GE,
                    )

                    # payload: region-local frame position = 2*(o+s) + oM
                    nc.vector.tensor_tensor(
                        out=PAY[:, o : o + hw], in0=iotaR[:, o : o + hw],
                        in1=oM[:, 0:hw], op=ADD,
                    )
                    # chained scan seed: 0 at region start, else prev count+1
                    if ci == 0:
                        seed = zero1[:, 0:1]
                    else:
                        jprev = reg["chunks"][ci - 1]
                        nc.scalar.activation(
                            r0[:, j : j + 1], cntf[:, jprev : jprev + 1],
                            COPY, bias=1.0,
                        )
                        seed = r0[:, j : j + 1]
                    nc.vector._custom_dve(
                        SCANIDX2, out=IDX[:, o : o + hw], in0=eM[:, 0:hw],
                        in1=oM[:, 0:hw], s0=-1.0, s1=seed,
                        accum_out=cntf[:, j : j + 1],
                    )
                    o += hw

                K = reg["K"]
                bkt = st_pool.tile([p, 1536], I16, tag="bkt")
                nc.gpsimd.local_scatter(
                    out_ap=bkt[:, 0:K], data_ap=PAY[:, 0:o], idxs_ap=IDX[:, 0:o],
                    channels=p, num_elems=K, num_idxs=o,
                )
                nc.sync.dma_start(stage[:, koff : koff + K], bkt[:, 0:K])
                koff += K

            nc.sync.dma_start(counts[:], cntf[:])
    nc.compile()
    return nc


_cached = {}


def _get_nc():
    if "nc" not in _cached:
        _cached["nc"] = build_kernel()
    return _cached["nc"]


def _host_reference_fallback(x):
    """Exact numpy fallback (used only for adjacent-equal fp32 peak ties)."""
    n = x.shape[0]
    import numpy.lib.stride_tricks as st

    xp = np.pad(x, (3, 3), constant_values=-np.inf)
    pooled = st.sliding_window_view(xp, 7).max(axis=1)
    peak = (x == pooled) & (x > 0)
    idx = np.arange(n, dtype=np.int64)
    prev = np.concatenate([[False], peak[:-1]])
    is_new = peak & ~prev
    sec = np.cumsum(is_new) - 1
    sums = np.zeros(MAX_BEATS + 1, np.float64)
    cnts = np.zeros(MAX_BEATS + 1, np.float64)
    sel = peak & (sec < MAX_BEATS)
    np.add.at(sums, sec[sel], idx[sel].astype(np.float64))
    np.add.at(cnts, sec[sel], 1.0)
    out = np.full(MAX_BEATS, -1.0, np.float32)
    m = cnts[:MAX_BEATS] > 0
    out[m] = (sums[:MAX_BEATS][m] / cnts[:MAX_BEATS][m]).astype(np.float32)
    return out[None, :]


def kernel(logit: np.ndarray) -> np.ndarray:
    x = np.asarray(logit, dtype=np.float32)[0]

    # host guard: adjacent-equal fp32 window maxima need the exact path
    eq_next = x[:-1] == x[1:]
    if eq_next.any():
        cand = np.nonzero(eq_next)[0]
        cand = cand[(x[cand] > 0)]
        if cand.size:
            xp = np.pad(x, (3, 3), constant_values=-np.inf)
            for i in cand:
                if (
                    x[i] == xp[i : i + 7].max()
                    and x[i + 1] == xp[i + 1 : i + 8].max()
                ):
                    return _host_reference_fallback(x)

    nc = _get_nc()

    xpad = np.full(NFRAMES + 8, np.float32(-3.0e38), dtype=np.float32)
    xpad[4 : 4 + NFRAMES] = x

    in_maps = []
    for c in range(NCORES):
        base = c * PERCORE
        in_maps.append(
            {"xin": np.ascontiguousarray(xpad[base : base + PERCORE + HALO])}
        )

    global _last_in_maps
    _last_in_maps = in_maps
    res = bass_utils.run_bass_kernel_spmd(nc, in_maps, core_ids=list(range(NCORES)))

    # host: decode buckets -> candidate positions in global frame order
    # (per core: rows ascend; within a row: regions ascend)
    reg_start = [0, 8192, 14336]
    ordered = []
    for c in range(NCORES):
        stageo = res.results[c]["stage"].astype(np.int64)
        cnt = res.results[c]["counts"]
        row_parts = [[] for _ in range(P)]
        koff = 0
        for ri, reg in enumerate(REGIONS):
            K = reg["K"]
            S = stageo[:, koff : koff + K]
            koff += K
            C = np.clip(cnt[:, reg["chunks"][-1]].astype(np.int64) + 1, 0, K)
            nxt = np.roll(S, -1, axis=1)
            hole = S == 0
            hole[:, 0] &= nxt[:, 0] == 1
            S = np.where(hole, nxt - 1, S)
            for p_ in range(P):
                v = S[p_, : C[p_]] + (
                    c * PERCORE + p_ * W + reg_start[ri]
                )
                row_parts[p_].append(v)
        for p_ in range(P):
            ordered.extend(row_parts[p_])
    cand = np.concatenate(ordered)

    # exact fp32 verification of every candidate (removes bf16 ties)
    xg = np.pad(x, (3, 3), constant_values=-np.float32(np.inf))
    win = xg[cand[:, None] + np.arange(7)[None, :]]
    xv = x[cand]
    keep = (xv >= win.max(axis=1)) & (xv > 0)
    beats = cand[keep][:MAX_BEATS]

    out = np.full(MAX_BEATS, -1.0, dtype=np.float32)
    out[: beats.size] = beats.astype(np.float32)
    return out[None, :]


# revision 7
# speedup vs baseline: 1.6974x; 1.6741x over previous
"""Trainium2 Bass kernel for nn_MinimalBeatDecoder (nms_detection), v3.

Reference semantics: peaks = positive local maxima of a 7-wide window over a
16.7M-frame logit stream; output = frame index of each peak (sections are
single peaks in the no-tie case), first 2^21 of them, padded with -1.

Per core (2^21 frames as 128 rows x 16384), chunked [512, 1536, 2048 x 7]:
  - ACT engine casts each fp32 chunk into two dense bf16 relu planes
    (even/odd frames) via strided activation ops. relu folds the x > 0
    test into the window max; relu+bf16 rounding is monotone, so the bf16
    comparisons yield a candidate-peak SUPERSET of the true fp32 peaks
    (bf16 ties and all-nonpositive plateaus add ~0.5% false candidates).
  - DVE computes the 7-window candidate mask on the planes with 7 dense
    2x-mode tensor_tensor ops per chunk:
      P[s]  = max(xe[s], xo[s])            pair max
      R[s]  = max(P[s-1], P[s+1])
      Q[s]  = max(R[s], P[s])              6-window max
      eM[s] = xe[s] >= max(Q[s], xo[s-2])  even-parity candidates
      oM[s] = xo[s] >= max(Q[s], xe[s+2])  odd-parity candidates
  - the masks stream straight back to DRAM (no on-device compaction: the
    only compaction engine, GPSIMD LocalScatter, runs at ~3ns/idx and
    starves the DVE while active, costing more than it saves).
  - host: decode masks to ordered candidate positions (vectorized, at most
    2 per pair), exact fp32 verification of every candidate (vectorized
    7-window max at candidate positions) removes the false candidates.

An exact numpy fallback handles inputs with adjacent-equal fp32 peak ties
(reference merges those into averaged sections; gaussian inputs never tie).
"""

import sys

sys.path.insert(0, "/opt/trn_rl_repo")

import numpy as np

import concourse.bacc as bacc
import concourse.bass as bass
import concourse.mybir as mybir
import concourse.tile as tile
from concourse import bass_utils

# geometry
NCORES = 8
NFRAMES = 16_777_216
PERCORE = NFRAMES // NCORES  # 2^21
MAX_BEATS = NFRAMES // 8  # 2^21

P = 128
W = PERCORE // P  # 16384 frames per row
WP = W // 2  # 8192 pairs per row
HALO = 8

CHUNKS = [512, 1536, 2048, 2048, 2048, 2048, 2048, 2048, 2048]

F32 = mybir.dt.float32
BF16 = mybir.dt.bfloat16
I16 = mybir.dt.int16


def build_kernel(p=P, w=W):
    """Per-core SPMD program. Inputs:
      xin [p*w + HALO] f32  (frame t of this core at index t+4)
    Outputs:
      me [p, WP] i16  (even-parity candidate mask, 1 bit per pair)
      mo [p, WP] i16  (odd-parity candidate mask)
    """
    nc = bacc.Bacc("TRN2", target_bir_lowering=False)
    xin = nc.dram_tensor("xin", [p * w + HALO], F32, kind="ExternalInput")
    me_d = nc.dram_tensor("me", [p, WP], I16, kind="ExternalOutput")
    mo_d = nc.dram_tensor("mo", [p, WP], I16, kind="ExternalOutput")

    MX = mybir.AluOpType.max
    GE = mybir.AluOpType.is_ge
    RELU = mybir.ActivationFunctionType.Relu

    with tile.TileContext(nc) as tc:
        with (
            tc.tile_pool(name="io", bufs=3) as io_pool,
            tc.tile_pool(name="pl", bufs=3) as pl_pool,
            tc.tile_pool(name="wk", bufs=3) as wk_pool,
        ):
            o = 0  # pair offset within row
            for j, cw in enumerate(CHUNKS):
                hw = cw // 2
                off = 2 * o
                xh = io_pool.tile([p, 2056], F32, tag="xh")
                src = bass.AP(tensor=xin, offset=off, ap=[[w, p], [1, cw + 8]])
                nc.sync.dma_start(xh[:, 0 : cw + 8], src)

                # relu bf16 planes: xet[k]=relu(x[2(k-2)]), xot[k]=relu(x[2(k-2)+1])
                xet = pl_pool.tile([p, 1028], BF16, tag="xet")
                xot = pl_pool.tile([p, 1028], BF16, tag="xot")
                nc.scalar.activation(
                    xet[:, 0 : hw + 4], xh[:, 0 : cw + 8 : 2], RELU, bias=0.0
                )
                nc.scalar.activation(
                    xot[:, 0 : hw + 4], xh[:, 1 : cw + 8 : 2], RELU, bias=0.0
                )

                Pt = wk_pool.tile([p, 1028], BF16, tag="Pt")
                nc.vector.tensor_tensor(
                    out=Pt[:, 0 : hw + 4], in0=xet[:, 0 : hw + 4],
                    in1=xot[:, 0 : hw + 4], op=MX,
                )
                Rt = wk_pool.tile([p, 1024], BF16, tag="Rt")
                nc.vector.tensor_tensor(
                    out=Rt[:, 0:hw], in0=Pt[:, 1 : hw + 1],
                    in1=Pt[:, 3 : hw + 3], op=MX,
                )
                Qt = wk_pool.tile([p, 1024], BF16, tag="Qt")
                nc.vector.tensor_tensor(
                    out=Qt[:, 0:hw], in0=Rt[:, 0:hw], in1=Pt[:, 2 : hw + 2],
                    op=MX,
                )
                eW = wk_pool.tile([p, 1024], BF16, tag="eW")
                nc.vector.tensor_tensor(
                    out=eW[:, 0:hw], in0=Qt[:, 0:hw], in1=xot[:, 0:hw], op=MX
                )
                eM = wk_pool.tile([p, 1024], I16, tag="eM")
                nc.vector.tensor_tensor(
                    out=eM[:, 0:hw], in0=xet[:, 2 : hw + 2], in1=eW[:, 0:hw],
                    op=GE,
                )
                oW = wk_pool.tile([p, 1024], BF16, tag="oW")
                nc.vector.tensor_tensor(
                    out=oW[:, 0:hw], in0=Qt[:, 0:hw], in1=xet[:, 4 : hw + 4],
                    op=MX,
                )
                oM = wk_pool.tile([p, 1024], I16, tag="oM")
                nc.vector.tensor_tensor(
                    out=oM[:, 0:hw], in0=xot[:, 2 : hw + 2], in1=oW[:, 0:hw],
                    op=GE,
                )

                nc.sync.dma_start(me_d[:, o : o + hw], eM[:, 0:hw])
                nc.sync.dma_start(mo_d[:, o : o + hw], oM[:, 0:hw])
                o += hw
    nc.compile()
    return nc


_cached = {}


def _get_nc():
    if "nc" not in _cached:
        _cached["nc"] = build_kernel()
    return _cached["nc"]


def _host_reference_fallback(x):
    """Exact numpy fallback (used only for adjacent-equal fp32 peak ties)."""
    n = x.shape[0]
    import numpy.lib.stride_tricks as st

    xp = np.pad(x, (3, 3), constant_values=-np.inf)
    pooled = st.sliding_window_view(xp, 7).max(axis=1)
    peak = (x == pooled) & (x > 0)
    idx = np.arange(n, dtype=np.int64)
    prev = np.concatenate([[False], peak[:-1]])
    is_new = peak & ~prev
    sec = np.cumsum(is_new) - 1
    sums = np.zeros(MAX_BEATS + 1, np.float64)
    cnts = np.zeros(MAX_BEATS + 1, np.float64)
    sel = peak & (sec < MAX_BEATS)
    np.add.at(sums, sec[sel], idx[sel].astype(np.float64))
    np.add.at(cnts, sec[sel], 1.0)
    out = np.full(MAX_BEATS, -1.0, np.float32)
    m = cnts[:MAX_BEATS] > 0
    out[m] = (sums[:MAX_BEATS][m] / cnts[:MAX_BEATS][m]).astype(np.float32)
    return out[None, :]


def kernel(logit: np.ndarray) -> np.ndarray:
    x = np.asarray(logit, dtype=np.float32)[0]

    # host guard: adjacent-equal fp32 window maxima need the exact path
    eq_next = x[:-1] == x[1:]
    if eq_next.any():
        cand = np.nonzero(eq_next)[0]
        cand = cand[(x[cand] > 0)]
        if cand.size:
            xp = np.pad(x, (3, 3), constant_values=-np.inf)
            for i in cand:
                if (
                    x[i] == xp[i : i + 7].max()
                    and x[i + 1] == xp[i + 1 : i + 8].max()
                ):
                    return _host_reference_fallback(x)

    nc = _get_nc()

    xpad = np.full(NFRAMES + 8, np.float32(-3.0e38), dtype=np.float32)
    xpad[4 : 4 + NFRAMES] = x

    in_maps = []
    for c in range(NCORES):
        base = c * PERCORE
        in_maps.append(
            {"xin": np.ascontiguousarray(xpad[base : base + PERCORE + HALO])}
        )

    global _last_in_maps
    _last_in_maps = in_maps
    res = bass_utils.run_bass_kernel_spmd(nc, in_maps, core_ids=list(range(NCORES)))

    # host: masks -> ordered candidate positions (<= 2 per pair, even first)
    em = np.concatenate([res.results[c]["me"].reshape(-1) for c in range(NCORES)])
    om = np.concatenate([res.results[c]["mo"].reshape(-1) for c in range(NCORES)])
    v = em + 2 * om  # flat pair index == global pair (row-major == frame order)
    nz = np.flatnonzero(v)
    vv = v[nz]
    both = vv == 3
    ncand = nz.size + int(both.sum())
    first = 2 * nz + (vv == 2)  # even position unless odd-only
    starts = np.cumsum(1 + both) - (1 + both)
    cand = np.empty(ncand, dtype=np.int64)
    cand[starts] = first
    cand[starts[both] + 1] = 2 * nz[both] + 1

    # exact fp32 verification of every candidate (removes bf16/relu ties)
    xg = np.pad(x, (3, 3), constant_values=-np.float32(np.inf))
    win = xg[cand[:, None] + np.arange(7)[None, :]]
    xv = x[cand]
    keep = (xv >= win.max(axis=1)) & (xv > 0)
    beats = cand[keep][:MAX_BEATS]

    out = np.full(MAX_BEATS, -1.0, dtype=np.float32)
    out[: beats.size] = beats.astype(np.float32)
    return out[None, :]
